# revision 14
# baseline (speedup 1.0000x reference)
"""AtariGRU Trainium2 kernel: B=256,T=128,D=U=512, Keras GRU (reset_after) with
done/step-counter resets. Data-parallel over batch on 8 cores; time recurrence
parallelized over T-chunks (len 8 + halo 16) exploiting the guaranteed reset
every <=16 steps. 4 chunk-groups of 128 rows pipeline PE against the gate
chain. bf16 matmuls, f32 PSUM accumulate.

kernel(**inputs) takes FULL inputs, returns (sequences, state_f, step_f).
"""
import sys
import numpy as np

for _p in ("/opt/trn_rl_repo", "/root/.axon_site/_ro/trn_rl_repo"):
    if _p not in sys.path:
        sys.path.insert(0, _p)

import ml_dtypes

BF16 = ml_dtypes.bfloat16

# problem constants (hardcoded per harness rules)
B, T, D, UN = 256, 128, 512, 512
NCORES = 8
BC = B // NCORES          # 32 batch rows per core
CH = 8                    # chunk length
HALO = 16                 # halo length (max reset gap)
S = CH + HALO             # 24 steps per chunk
NCHUNK = T // CH          # 16 chunks
NGRP = 4                  # chunk-groups of 4 chunks x 32 rows = 128 partitions
CPG = NCHUNK // NGRP      # 4 chunks per group
G3 = 3 * UN               # 1536
MEMORY_SIZE = 16

_nc_cache = {}


def _build_nc():
    import concourse.bacc as bacc
    import concourse.mybir as mybir
    import concourse.tile as tile
    from contextlib import ExitStack

    dt = mybir.dt
    AF = mybir.ActivationFunctionType
    f32 = dt.float32
    bf = dt.bfloat16

    nc = bacc.Bacc(None, target_bir_lowering=False)

    xT_d = nc.declare_dram_parameter("xT", [NGRP, CH, 128, 512], bf, isOutput=False)
    W_d = nc.declare_dram_parameter("Wm", [D, G3], bf, isOutput=False)
    U_d = nc.declare_dram_parameter("Um", [UN, G3], bf, isOutput=False)
    bx_d = nc.declare_dram_parameter("bias_x", [128, G3], bf, isOutput=False)
    bh_d = nc.declare_dram_parameter("bias_h", [128, UN], bf, isOutput=False)
    mk_d = nc.declare_dram_parameter("mask", [NGRP, 128, S], f32, isOutput=False)
    h0_d = nc.declare_dram_parameter("h0pad", [NGRP, 128, UN], bf, isOutput=False)
    out_d = nc.declare_dram_parameter("out", [BC, T, UN], bf, isOutput=True)

    with ExitStack() as ctx:
        tc = ctx.enter_context(tile.TileContext(nc))
        const = ctx.enter_context(tc.tile_pool(name="const", bufs=1))
        xmp = ctx.enter_context(tc.tile_pool(name="xmp", bufs=1))
        work = ctx.enter_context(tc.tile_pool(name="work", bufs=2))
        xtp = ctx.enter_context(tc.tile_pool(name="xtp", bufs=8))
        psx = ctx.enter_context(tc.tile_pool(name="psx", bufs=2, space="PSUM"))
        psr = ctx.enter_context(tc.tile_pool(name="psr", bufs=1, space="PSUM"))

        # ---- constants into SBUF ----
        U_sb = []
        W_sb = []
        for k in range(4):
            ut = const.tile([128, G3], bf, tag=f"U{k}")
            nc.sync.dma_start(out=ut[:], in_=U_d[k * 128:(k + 1) * 128, :])
            U_sb.append(ut)
            wt = const.tile([128, G3], bf, tag=f"W{k}")
            nc.sync.dma_start(out=wt[:], in_=W_d[k * 128:(k + 1) * 128, :])
            W_sb.append(wt)
        bx_sb = const.tile([128, G3], bf, tag="bx")
        nc.sync.dma_start(out=bx_sb[:], in_=bx_d[:, :])
        bh_sb = const.tile([128, UN], bf, tag="bh")
        nc.sync.dma_start(out=bh_sb[:], in_=bh_d[:, :])
        mk_sb = []
        for g in range(NGRP):
            mt = const.tile([128, S], f32, tag=f"mk{g}")
            nc.sync.dma_start(out=mt[:], in_=mk_d[g, :, :])
            mk_sb.append(mt)
        h0g0 = const.tile([128, UN], bf, tag="h0g0")
        nc.sync.dma_start(out=h0g0[:], in_=h0_d[0, :, :])

        # shifted identities: sel(base)[p,q] = 1 iff q == p + base
        def make_ident(tag, base):
            t = const.tile([128, 128], bf, tag=tag)
            nc.gpsimd.memset(t[:], 0.0)
            nc.gpsimd.affine_select(
                out=t[:], in_=t[:],
                compare_op=mybir.AluOpType.not_equal,
                fill=1.0, base=base,
                pattern=[[-1, 128]], channel_multiplier=1,
            )
            return t

        ident = make_ident("ident", 0)
        ishA = make_ident("ishA", 64)    # q = p+64   (own group, j<8)
        ishB = make_ident("ishB", 32)    # q = p+32   (own group, 8<=j<16)
        icrA = make_ident("icrA", -64)   # q = p-64   (prev group, j<8)
        icrB = make_ident("icrB", -96)   # q = p-96   (prev group, 8<=j<16)

        out_v = out_d[:, :, :].rearrange("b (c t) d -> c b t d", c=NCHUNK)

        # ---- phase 1: xm[g] = X@W + bias_x, layout [p=(c,b), (tl,1536)] ----
        xm_sb = []
        for g in range(NGRP):
            xm = xmp.tile([128, CH * G3], bf, tag=f"xm{g}")
            xm_sb.append(xm)
        for g in range(NGRP):
            for tl in range(CH):
                xt = xtp.tile([128, 512], bf, tag="xt")
                nc.sync.dma_start(out=xt[:], in_=xT_d[g, tl, :, :])
                for npart in range(3):
                    ps = psx.tile([128, 512], f32, tag="psx")
                    for k in range(4):
                        nc.tensor.matmul(
                            ps[:], lhsT=xt[:, k * 128:(k + 1) * 128],
                            rhs=W_sb[k][:, npart * 512:(npart + 1) * 512],
                            start=(k == 0), stop=False,
                        )
                    nc.tensor.matmul(
                        ps[:], lhsT=ident[:],
                        rhs=bx_sb[:, npart * 512:(npart + 1) * 512],
                        start=False, stop=True,
                    )
                    nc.scalar.activation(
                        xm_sb[g][:, (tl * 3 + npart) * 512:(tl * 3 + npart + 1) * 512],
                        ps[:], AF.Copy,
                    )

        # ---- phase 2: recurrence, 24 steps x 4 groups of 128 rows ----
        hT = []
        carry = []
        for g in range(NGRP):
            t = work.tile([128, UN], bf, tag=f"hT{g}")
            nc.vector.memset(t[:], 0.0)
            hT.append(t)
            c = work.tile([128, UN], bf, tag=f"carry{g}")
            nc.vector.memset(c[:], 0.0)
            carry.append(c)

        for j in range(S):
            main = j >= HALO
            # halo sub-case: j<8 reads 2 chunks back (tl=j); 8<=j<16 reads
            # 1 chunk back (tl=j-8)
            if main:
                xoff = (j - HALO) * G3
            elif j < CH:
                xoff = j * G3
            else:
                xoff = (j - CH) * G3
            Iown = ident if main else (ishA if j < CH else ishB)
            Icross = None if main else (icrA if j < CH else icrB)
            for g in range(NGRP):
                ptag = f"P{g % 2}"
                cross = (Icross is not None and g > 0)
                P = psr.tile([128, G3], f32, tag=ptag)
                # hm = h @ U  (12 matmuls, k-major)
                for k in range(4):
                    for npart in range(3):
                        nc.tensor.matmul(
                            P[:, npart * 512:(npart + 1) * 512],
                            lhsT=hT[g][:, k * 128:(k + 1) * 128],
                            rhs=U_sb[k][:, npart * 512:(npart + 1) * 512],
                            start=(k == 0), stop=False,
                        )
                # fold xm for z,r parts (shifted during halo, + prev-group part)
                for npart in range(2):
                    sl = slice(xoff + npart * 512, xoff + (npart + 1) * 512)
                    nc.tensor.matmul(
                        P[:, npart * 512:(npart + 1) * 512],
                        lhsT=Iown[:], rhs=xm_sb[g][:, sl],
                        start=False, stop=not cross,
                    )
                    if cross:
                        nc.tensor.matmul(
                            P[:, npart * 512:(npart + 1) * 512],
                            lhsT=Icross[:], rhs=xm_sb[g - 1][:, sl],
                            start=False, stop=True,
                        )
                # bias_h fold for the h part
                nc.tensor.matmul(
                    P[:, 1024:1536], lhsT=ident[:], rhs=bh_sb[:],
                    start=False, stop=True,
                )

                # gates
                z = work.tile([128, UN], bf, tag="z", bufs=5)
                nc.scalar.activation(z[:], P[:, 0:512], AF.Sigmoid)
                r = work.tile([128, UN], bf, tag="r", bufs=5)
                nc.scalar.activation(r[:], P[:, 512:1024], AF.Sigmoid)
                hh = work.tile([128, UN], bf, tag="hh", bufs=5)
                nc.scalar.activation(hh[:], P[:, 1024:1536], AF.Copy)
                rh = work.tile([128, UN], bf, tag="rh", bufs=5)
                nc.vector.tensor_mul(rh[:], r[:], hh[:])

                if main:
                    xmh_ap = xm_sb[g][:, xoff + 1024:xoff + 1536]
                else:
                    xmh_ps = psr.tile([128, 512], f32, tag=ptag)
                    sl = slice(xoff + 1024, xoff + 1536)
                    nc.tensor.matmul(
                        xmh_ps[:], lhsT=Iown[:], rhs=xm_sb[g][:, sl],
                        start=True, stop=not cross,
                    )
                    if cross:
                        nc.tensor.matmul(
                            xmh_ps[:], lhsT=Icross[:], rhs=xm_sb[g - 1][:, sl],
                            start=False, stop=True,
                        )
                    xmh_ap = xmh_ps[:]

                hcpre = work.tile([128, UN], bf, tag="hcpre", bufs=5)
                nc.vector.tensor_add(hcpre[:], rh[:], xmh_ap)
                hc = work.tile([128, UN], bf, tag="hc", bufs=5)
                nc.scalar.activation(hc[:], hcpre[:], AF.Tanh)
                dd = work.tile([128, UN], bf, tag="dd", bufs=5)
                nc.vector.tensor_sub(dd[:], carry[g][:], hc[:])
                zd = work.tile([128, UN], bf, tag="zd", bufs=5)
                nc.vector.tensor_mul(zd[:], z[:], dd[:])
                hnew = work.tile([128, UN], bf, tag="hnew", bufs=6)
                nc.vector.tensor_add(hnew[:], hc[:], zd[:])
                cnew = work.tile([128, UN], bf, tag=f"carry{g}")
                nc.vector.tensor_scalar_mul(cnew[:], hnew[:], mk_sb[g][:, j:j + 1])
                if j == HALO - 1 and g == 0:
                    c2 = work.tile([128, UN], bf, tag="cinj")
                    nc.vector.tensor_add(c2[:], cnew[:], h0g0[:])
                    cnew = c2
                carry[g] = cnew

                if main:
                    nc.gpsimd.dma_start(
                        out=out_v[g * CPG:(g + 1) * CPG, :, j - HALO, :],
                        in_=hnew[:],
                    )
                if j < S - 1:
                    hTn = work.tile([128, UN], bf, tag=f"hT{g}")
                    for k in range(4):
                        nc.sync.dma_start_transpose(
                            hTn[:, k * 128:(k + 1) * 128],
                            cnew[:, k * 128:(k + 1) * 128],
                        )
                    hT[g] = hTn

    nc.compile()
    return nc


def _host_scan(dones, step0):
    """reset mask r[B,T] (1=reset after step t) and final step counter."""
    Bn, Tn = dones.shape
    s = step0[:, 0].astype(np.int64).copy()
    r = np.zeros((Bn, Tn), dtype=bool)
    for t in range(Tn):
        s_new = s + 1
        reset = (dones[:, t] == 1) | (s_new % MEMORY_SIZE == 0)
        r[:, t] = reset
        s = np.where(reset, 0, s_new)
    return r, s.astype(np.int32)[:, None]


def _make_in_maps(inputs, dones, state, step, W, U, b):
    r, step_f = _host_scan(dones, step)
    mask_f = (~r).astype(np.float32)          # [B, T]

    # bias prep: xm gets b0+b1 for z,r parts and b0 only for h part;
    # bias_h (= b1 h-part) folded into hm during recurrence.
    bx = b[0].copy()                          # [3U]
    bx[:2 * UN] += b[1][:2 * UN]
    bias_x = np.broadcast_to(bx.astype(BF16), (128, G3)).copy()
    bias_h = np.broadcast_to(b[1][2 * UN:].astype(BF16), (128, UN)).copy()

    Wb = W.astype(BF16)
    Ub = U.astype(BF16)
    xb = inputs.astype(BF16)                  # [B, T, D]

    # per-(g,p,j) mask, vectorized: t = (g*CPG+cl)*CH - HALO + j
    tgrid = ((np.arange(NGRP)[:, None, None] * CPG +
              np.arange(CPG)[None, :, None]) * CH - HALO +
             np.arange(S)[None, None, :])     # [NGRP, CPG, S]
    valid = (tgrid >= 0)
    tclip = np.clip(tgrid, 0, T - 1)

    in_maps = []
    for ci in range(NCORES):
        bs = slice(ci * BC, (ci + 1) * BC)
        m_core = mask_f[bs]                   # [32, T]
        mk = m_core[:, tclip] * valid[None]   # [32, NGRP, CPG, S]
        mk = np.ascontiguousarray(
            mk.transpose(1, 2, 0, 3).reshape(NGRP, 128, S)).astype(np.float32)
        h0p = np.zeros((NGRP, 128, UN), dtype=BF16)
        h0p[0, 0:32, :] = state[bs].astype(BF16)
        xs = xb[bs].reshape(BC, NCHUNK, CH, D)      # [b, c, tl, d]
        xTc = np.zeros((NGRP, CH, 128, 512), dtype=BF16)
        for g in range(NGRP):
            arr = xs[:, g * CPG:(g + 1) * CPG]      # [b, cl, tl, d]
            a2 = arr.transpose(2, 3, 1, 0)          # [tl, d, cl, b]
            a3 = a2.reshape(CH, 4, 128, CPG, 32)    # [tl, k, dk, cl, b]
            xTc[g] = a3.transpose(0, 2, 1, 3, 4).reshape(CH, 128, 512)
        in_maps.append({
            "xT": xTc,
            "Wm": Wb, "Um": Ub,
            "bias_x": bias_x, "bias_h": bias_h,
            "mask": mk, "h0pad": h0p,
        })
    return in_maps, mask_f, step_f


def kernel(inputs, dones, state, step, W, U, b):
    from concourse.bass_utils import run_bass_kernel_spmd

    if "nc" not in _nc_cache:
        _nc_cache["nc"] = _build_nc()
    nc = _nc_cache["nc"]

    in_maps, mask_f, step_f = _make_in_maps(inputs, dones, state, step, W, U, b)
    res = run_bass_kernel_spmd(nc, in_maps, core_ids=list(range(NCORES)))
    seq = np.concatenate(
        [res.results[ci]["out"].astype(np.float32) for ci in range(NCORES)], axis=0
    )                                          # [B, T, U]
    state_f = seq[:, -1, :] * mask_f[:, -1:]
    return seq, state_f, step_f


# revision 15
# speedup vs baseline: 1.2917x; 1.2917x over previous
"""AtariGRU Trainium2 kernel: B=256,T=128,D=U=512, Keras GRU (reset_after) with
done/step-counter resets. Data-parallel over batch on 8 cores; time recurrence
parallelized over T-chunks (len 8 + halo 16) exploiting the guaranteed reset
every <=16 steps. 4 chunk-groups of 128 rows pipeline PE against the gate
chain. bf16 matmuls, f32 PSUM accumulate.

kernel(**inputs) takes FULL inputs, returns (sequences, state_f, step_f).
"""
import sys
import numpy as np

for _p in ("/opt/trn_rl_repo", "/root/.axon_site/_ro/trn_rl_repo"):
    if _p not in sys.path:
        sys.path.insert(0, _p)

import ml_dtypes

BF16 = ml_dtypes.bfloat16

# problem constants (hardcoded per harness rules)
B, T, D, UN = 256, 128, 512, 512
NCORES = 8
BC = B // NCORES          # 32 batch rows per core
CH = 8                    # chunk length
HALO = 16                 # halo length (max reset gap)
S = CH + HALO             # 24 steps per chunk
NCHUNK = T // CH          # 16 chunks
NGRP = 4                  # chunk-groups of 4 chunks x 32 rows = 128 partitions
CPG = NCHUNK // NGRP      # 4 chunks per group
G3 = 3 * UN               # 1536
MEMORY_SIZE = 16

_nc_cache = {}


def _build_nc():
    import concourse.bacc as bacc
    import concourse.mybir as mybir
    import concourse.tile as tile
    from contextlib import ExitStack

    dt = mybir.dt
    AF = mybir.ActivationFunctionType
    f32 = dt.float32
    bf = dt.bfloat16

    nc = bacc.Bacc(None, target_bir_lowering=False)

    xT_d = nc.declare_dram_parameter("xT", [NGRP, CH, 128, 512], bf, isOutput=False)
    W_d = nc.declare_dram_parameter("Wm", [D, G3], bf, isOutput=False)
    U_d = nc.declare_dram_parameter("Um", [UN, G3], bf, isOutput=False)
    bx_d = nc.declare_dram_parameter("bias_x", [128, G3], bf, isOutput=False)
    bh_d = nc.declare_dram_parameter("bias_h", [128, UN], bf, isOutput=False)
    mk_d = nc.declare_dram_parameter("mask", [NGRP, 128, S], f32, isOutput=False)
    h0_d = nc.declare_dram_parameter("h0pad", [NGRP, 128, UN], bf, isOutput=False)
    out_d = nc.declare_dram_parameter("out", [BC, T, UN], bf, isOutput=True)

    with ExitStack() as ctx:
        tc = ctx.enter_context(tile.TileContext(nc))
        const = ctx.enter_context(tc.tile_pool(name="const", bufs=1))
        xmp = ctx.enter_context(tc.tile_pool(name="xmp", bufs=1))
        work = ctx.enter_context(tc.tile_pool(name="work", bufs=2))
        xtp = ctx.enter_context(tc.tile_pool(name="xtp", bufs=8))
        psx = ctx.enter_context(tc.tile_pool(name="psx", bufs=2, space="PSUM"))
        psr = ctx.enter_context(tc.tile_pool(name="psr", bufs=1, space="PSUM"))

        # ---- constants into SBUF ----
        U_sb = []
        W_sb = []
        for k in range(4):
            ut = const.tile([128, G3], bf, tag=f"U{k}")
            nc.sync.dma_start(out=ut[:], in_=U_d[k * 128:(k + 1) * 128, :])
            U_sb.append(ut)
            wt = const.tile([128, G3], bf, tag=f"W{k}")
            nc.sync.dma_start(out=wt[:], in_=W_d[k * 128:(k + 1) * 128, :])
            W_sb.append(wt)
        bx_sb = const.tile([128, G3], bf, tag="bx")
        nc.sync.dma_start(out=bx_sb[:], in_=bx_d[:, :])
        bh_sb = const.tile([128, UN], bf, tag="bh")
        nc.sync.dma_start(out=bh_sb[:], in_=bh_d[:, :])
        mk_sb = []
        for g in range(NGRP):
            mt = const.tile([128, S], f32, tag=f"mk{g}")
            nc.sync.dma_start(out=mt[:], in_=mk_d[g, :, :])
            mk_sb.append(mt)
        h0g0 = const.tile([128, UN], bf, tag="h0g0")
        nc.sync.dma_start(out=h0g0[:], in_=h0_d[0, :, :])

        # shifted identities: sel(base)[p,q] = 1 iff q == p + base
        def make_ident(tag, base):
            t = const.tile([128, 128], bf, tag=tag)
            nc.gpsimd.memset(t[:], 0.0)
            nc.gpsimd.affine_select(
                out=t[:], in_=t[:],
                compare_op=mybir.AluOpType.not_equal,
                fill=1.0, base=base,
                pattern=[[-1, 128]], channel_multiplier=1,
            )
            return t

        ident = make_ident("ident", 0)
        nbs = sorted({(HALO - 1 - j) // CH + 1 for j in range(HALO)})
        ish = {32 * nb: make_ident(f"ish{nb}", 32 * nb) for nb in nbs}
        icr = {32 * nb - 128: make_ident(f"icr{nb}", 32 * nb - 128) for nb in nbs}

        out_v = out_d[:, :, :].rearrange("b (c t) d -> c b t d", c=NCHUNK)

        # ---- phase 1: xm[g] = X@W + bias_x, layout [p=(c,b), (tl,1536)] ----
        xm_sb = []
        for g in range(NGRP):
            xm = xmp.tile([128, CH * G3], bf, tag=f"xm{g}")
            xm_sb.append(xm)
        for g in range(NGRP):
            for tl in range(CH):
                xt = xtp.tile([128, 512], bf, tag="xt")
                nc.sync.dma_start(out=xt[:], in_=xT_d[g, tl, :, :])
                for npart in range(3):
                    ps = psx.tile([128, 512], f32, tag="psx")
                    for k in range(4):
                        nc.tensor.matmul(
                            ps[:], lhsT=xt[:, k * 128:(k + 1) * 128],
                            rhs=W_sb[k][:, npart * 512:(npart + 1) * 512],
                            start=(k == 0), stop=False,
                        )
                    nc.tensor.matmul(
                        ps[:], lhsT=ident[:],
                        rhs=bx_sb[:, npart * 512:(npart + 1) * 512],
                        start=False, stop=True,
                    )
                    nc.scalar.activation(
                        xm_sb[g][:, (tl * 3 + npart) * 512:(tl * 3 + npart + 1) * 512],
                        ps[:], AF.Copy,
                    )

        # ---- phase 2: recurrence, 24 steps x 4 groups of 128 rows ----
        hT = []
        carry = []
        for g in range(NGRP):
            t = work.tile([128, UN], bf, tag=f"hT{g}")
            nc.vector.memset(t[:], 0.0)
            hT.append(t)
            c = work.tile([128, UN], bf, tag=f"carry{g}")
            nc.vector.memset(c[:], 0.0)
            carry.append(c)

        for j in range(S):
            main = j >= HALO
            # halo: step j reads nb chunks back at local offset tl'
            if main:
                xoff = (j - HALO) * G3
                Iown, Icross = ident, None
            else:
                nb = (HALO - 1 - j) // CH + 1
                xoff = (j - HALO + nb * CH) * G3
                Iown = ish[32 * nb]
                Icross = icr[32 * nb - 128]
            for g in range(NGRP):
                ptag = f"P{g % 2}"
                cross = (Icross is not None and g > 0)
                P = psr.tile([128, G3], f32, tag=ptag)
                # hm = h @ U  (12 matmuls, k-major)
                for k in range(4):
                    for npart in range(3):
                        nc.tensor.matmul(
                            P[:, npart * 512:(npart + 1) * 512],
                            lhsT=hT[g][:, k * 128:(k + 1) * 128],
                            rhs=U_sb[k][:, npart * 512:(npart + 1) * 512],
                            start=(k == 0), stop=False,
                        )
                # fold xm for z,r parts (shifted during halo, + prev-group part)
                for npart in range(2):
                    sl = slice(xoff + npart * 512, xoff + (npart + 1) * 512)
                    nc.tensor.matmul(
                        P[:, npart * 512:(npart + 1) * 512],
                        lhsT=Iown[:], rhs=xm_sb[g][:, sl],
                        start=False, stop=not cross,
                    )
                    if cross:
                        nc.tensor.matmul(
                            P[:, npart * 512:(npart + 1) * 512],
                            lhsT=Icross[:], rhs=xm_sb[g - 1][:, sl],
                            start=False, stop=True,
                        )
                # bias_h fold for the h part
                nc.tensor.matmul(
                    P[:, 1024:1536], lhsT=ident[:], rhs=bh_sb[:],
                    start=False, stop=True,
                )

                # gates
                z = work.tile([128, UN], bf, tag="z", bufs=5)
                nc.scalar.activation(z[:], P[:, 0:512], AF.Sigmoid)
                r = work.tile([128, UN], bf, tag="r", bufs=5)
                nc.scalar.activation(r[:], P[:, 512:1024], AF.Sigmoid)
                hh = work.tile([128, UN], bf, tag="hh", bufs=5)
                nc.scalar.activation(hh[:], P[:, 1024:1536], AF.Copy)
                rh = work.tile([128, UN], bf, tag="rh", bufs=5)
                nc.vector.tensor_mul(rh[:], r[:], hh[:])

                if main:
                    xmh_ap = xm_sb[g][:, xoff + 1024:xoff + 1536]
                else:
                    xmh_ps = psr.tile([128, 512], f32, tag=ptag)
                    sl = slice(xoff + 1024, xoff + 1536)
                    nc.tensor.matmul(
                        xmh_ps[:], lhsT=Iown[:], rhs=xm_sb[g][:, sl],
                        start=True, stop=not cross,
                    )
                    if cross:
                        nc.tensor.matmul(
                            xmh_ps[:], lhsT=Icross[:], rhs=xm_sb[g - 1][:, sl],
                            start=False, stop=True,
                        )
                    xmh_ap = xmh_ps[:]

                hcpre = work.tile([128, UN], bf, tag="hcpre", bufs=5)
                nc.vector.tensor_add(hcpre[:], rh[:], xmh_ap)
                hc = work.tile([128, UN], bf, tag="hc", bufs=5)
                nc.scalar.activation(hc[:], hcpre[:], AF.Tanh)
                dd = work.tile([128, UN], bf, tag="dd", bufs=5)
                nc.vector.tensor_sub(dd[:], carry[g][:], hc[:])
                zd = work.tile([128, UN], bf, tag="zd", bufs=5)
                nc.vector.tensor_mul(zd[:], z[:], dd[:])
                hnew = work.tile([128, UN], bf, tag="hnew", bufs=6)
                nc.vector.tensor_add(hnew[:], hc[:], zd[:])
                cnew = work.tile([128, UN], bf, tag=f"carry{g}")
                nc.vector.tensor_scalar_mul(cnew[:], hnew[:], mk_sb[g][:, j:j + 1])
                if j == HALO - 1 and g == 0:
                    c2 = work.tile([128, UN], bf, tag="cinj")
                    nc.vector.tensor_add(c2[:], cnew[:], h0g0[:])
                    cnew = c2
                carry[g] = cnew

                if main:
                    nc.gpsimd.dma_start(
                        out=out_v[g * CPG:(g + 1) * CPG, :, j - HALO, :],
                        in_=hnew[:],
                    )
                if j < S - 1:
                    tp = psx.tile([128, 512], bf, tag="psx")
                    hTn = work.tile([128, UN], bf, tag=f"hT{g}")
                    for k in range(4):
                        nc.tensor.transpose(
                            tp[:, k * 128:(k + 1) * 128],
                            cnew[:, k * 128:(k + 1) * 128],
                            ident[:],
                        )
                        if k % 2 == 0:
                            nc.vector.tensor_copy(
                                hTn[:, k * 128:(k + 1) * 128],
                                tp[:, k * 128:(k + 1) * 128],
                            )
                        else:
                            nc.scalar.activation(
                                hTn[:, k * 128:(k + 1) * 128],
                                tp[:, k * 128:(k + 1) * 128],
                                AF.Copy,
                            )
                    hT[g] = hTn

    nc.compile()
    return nc


def _host_scan(dones, step0):
    """reset mask r[B,T] (1=reset after step t) and final step counter."""
    Bn, Tn = dones.shape
    s = step0[:, 0].astype(np.int64).copy()
    r = np.zeros((Bn, Tn), dtype=bool)
    for t in range(Tn):
        s_new = s + 1
        reset = (dones[:, t] == 1) | (s_new % MEMORY_SIZE == 0)
        r[:, t] = reset
        s = np.where(reset, 0, s_new)
    return r, s.astype(np.int32)[:, None]


def _make_in_maps(inputs, dones, state, step, W, U, b):
    r, step_f = _host_scan(dones, step)
    mask_f = (~r).astype(np.float32)          # [B, T]

    # bias prep: xm gets b0+b1 for z,r parts and b0 only for h part;
    # bias_h (= b1 h-part) folded into hm during recurrence.
    bx = b[0].copy()                          # [3U]
    bx[:2 * UN] += b[1][:2 * UN]
    bias_x = np.broadcast_to(bx.astype(BF16), (128, G3)).copy()
    bias_h = np.broadcast_to(b[1][2 * UN:].astype(BF16), (128, UN)).copy()

    Wb = W.astype(BF16)
    Ub = U.astype(BF16)
    xb = inputs.astype(BF16)                  # [B, T, D]

    # per-(g,p,j) mask, vectorized: t = (g*CPG+cl)*CH - HALO + j
    tgrid = ((np.arange(NGRP)[:, None, None] * CPG +
              np.arange(CPG)[None, :, None]) * CH - HALO +
             np.arange(S)[None, None, :])     # [NGRP, CPG, S]
    valid = (tgrid >= 0)
    tclip = np.clip(tgrid, 0, T - 1)

    in_maps = []
    for ci in range(NCORES):
        bs = slice(ci * BC, (ci + 1) * BC)
        m_core = mask_f[bs]                   # [32, T]
        mk = m_core[:, tclip] * valid[None]   # [32, NGRP, CPG, S]
        mk = np.ascontiguousarray(
            mk.transpose(1, 2, 0, 3).reshape(NGRP, 128, S)).astype(np.float32)
        h0p = np.zeros((NGRP, 128, UN), dtype=BF16)
        h0p[0, 0:32, :] = state[bs].astype(BF16)
        xs = xb[bs].reshape(BC, NCHUNK, CH, D)      # [b, c, tl, d]
        xTc = np.zeros((NGRP, CH, 128, 512), dtype=BF16)
        for g in range(NGRP):
            arr = xs[:, g * CPG:(g + 1) * CPG]      # [b, cl, tl, d]
            a2 = arr.transpose(2, 3, 1, 0)          # [tl, d, cl, b]
            a3 = a2.reshape(CH, 4, 128, CPG, 32)    # [tl, k, dk, cl, b]
            xTc[g] = a3.transpose(0, 2, 1, 3, 4).reshape(CH, 128, 512)
        in_maps.append({
            "xT": xTc,
            "Wm": Wb, "Um": Ub,
            "bias_x": bias_x, "bias_h": bias_h,
            "mask": mk, "h0pad": h0p,
        })
    return in_maps, mask_f, step_f


def kernel(inputs, dones, state, step, W, U, b):
    from concourse.bass_utils import run_bass_kernel_spmd

    if "nc" not in _nc_cache:
        _nc_cache["nc"] = _build_nc()
    nc = _nc_cache["nc"]

    in_maps, mask_f, step_f = _make_in_maps(inputs, dones, state, step, W, U, b)
    res = run_bass_kernel_spmd(nc, in_maps, core_ids=list(range(NCORES)))
    seq = np.concatenate(
        [res.results[ci]["out"].astype(np.float32) for ci in range(NCORES)], axis=0
    )                                          # [B, T, U]
    state_f = seq[:, -1, :] * mask_f[:, -1:]
    return seq, state_f, step_f
